# revision 14
# baseline (speedup 1.0000x reference)
"""Separable depthwise box filter (r=8, 'same' zero padding) on 8 trn2 cores.

Math: per (n, c) plane P (512x512), out = B @ P @ B where B is the symmetric
banded 512x512 matrix with B[i, j] = 1/(2r+1) for |i - j| <= r.  On the PE
(out = lhsT.T @ rhs):

  pass 1: Zt = matmul(lhsT=P,  rhs=B) = P.T @ B   (vertical filter, transposed)
  pass 2: Y  = matmul(lhsT=Zt, rhs=B) = Z  @ B    (horizontal filter, restored)

Both passes stream only the banded columns of B: the K-chunk of rows
[128a, 128a+128) of B has nonzero columns only in [128a-r, 128a+128+r).
PSUM's per-element has_written bit makes the overlapping column windows
accumulate while fresh columns overwrite, so each (M-chunk, K-chunk) pair is
a single matmul: 560 streamed columns per M-chunk instead of 2048.

Everything on-device is bf16 (fp32 matmul streams at 1/4 rate; the 2e-2
tolerance leaves ~40x headroom over bf16's quantization error).  The host
casts x -> bf16 and the bf16 result -> fp32.  PSUM accumulates in fp32; the
PSUM->SBUF evacuations (which also downcast) are spread over the DVE, ACT
and GpSimd engines.  The two passes are software-pipelined with a one-plane
skew (pass 1 of plane p runs on the PE while pass 2 of plane p-1's operand
tile is still being evacuated) so evacuation latency never bubbles the PE.

Sharding: batch dim (8) across the 8 cores; each core filters its 16 channel
planes independently (no cross-core communication).
"""

import numpy as np

_CACHE = {}

N_CORES = 8
P = 128
H = W = 512
A = H // P  # 4 row-chunks per plane


def _band_windows(r):
    """Nonzero column window [n0, n1) of B rows [128a, 128a+128), per a."""
    return [(max(0, P * a - r), min(W, P * a + P + r)) for a in range(A)]


def _build(r, n_planes):
    import concourse.mybir as mybir
    from concourse import bacc
    from concourse.tile import TileContext

    bf16 = mybir.dt.bfloat16
    f32 = mybir.dt.float32
    win = _band_windows(r)

    nc = bacc.Bacc()
    x_d = nc.declare_dram_parameter("x", [n_planes * H, W], bf16, isOutput=False)
    b_d = nc.declare_dram_parameter("b", [H, W], bf16, isOutput=False)
    y_d = nc.declare_dram_parameter("y", [n_planes * H, W], bf16, isOutput=True)

    x_ap = x_d.ap().rearrange("(p a q) n -> p q a n", p=n_planes, q=P)
    y_ap = y_d.ap().rearrange("(p a q) n -> p q a n", p=n_planes, q=P)
    b_ap = b_d.ap().rearrange("(a q) n -> q a n", q=P)

    with TileContext(nc) as tc:
        with (
            tc.tile_pool(name="bmat", bufs=1) as bpool,
            tc.tile_pool(name="xin", bufs=6) as xpool,
            tc.tile_pool(name="zmid", bufs=3) as zpool,
            tc.tile_pool(name="yout", bufs=6) as opool,
            tc.tile_pool(name="ps1", bufs=2, space="PSUM") as ps1,
            tc.tile_pool(name="ps2", bufs=2, space="PSUM") as ps2,
        ):
            bt = bpool.tile([P, A, W], bf16)
            xt0 = xpool.tile([P, A, W], bf16, name="xt0", tag="xt")
            # Interleave plane-0 x chunks with B chunks on the SP HWDGE ring
            # so the a=0 matmuls can start early; land the first matmul's
            # operands (B window 0 + x chunk 0) first.
            w0, w1 = win[0]
            nc.sync.dma_start(out=bt[:, 0, w0:w1], in_=b_ap[:, 0, w0:w1])
            nc.sync.dma_start(out=xt0[:, 0, :], in_=x_ap[0, :, 0, :])
            nc.sync.dma_start(out=bt[:, 0, w1:W], in_=b_ap[:, 0, w1:W])
            nc.sync.dma_start(out=xt0[:, 1, :], in_=x_ap[0, :, 1, :])
            nc.sync.dma_start(out=bt[:, 1, :], in_=b_ap[:, 1, :])
            nc.sync.dma_start(out=xt0[:, 2:4, :], in_=x_ap[0, :, 2:4, :])
            nc.sync.dma_start(out=bt[:, 2:4, :], in_=b_ap[:, 2:4, :])

            # Warm the PE while the input stream primes: the Tensor engine
            # needs ~3us of continuous execution to leave the low/mid
            # p-states (0.65/1.2 GHz) and reach 2.4 GHz.  Dummy matmuls on
            # the first-landed B chunk (result discarded) put the ramp time
            # behind us before the first real plane is ready.
            warm = ps1.tile([P, 2, W], f32, name="warm", tag="ps1")
            for _ in range(7):
                nc.tensor.matmul(
                    warm[:, 0, :],
                    bt[:, 0, 0:P],
                    bt[:, 0, :],
                    start=True,
                    stop=True,
                    skip_group_check=True,
                )

            zts = [None] * n_planes

            # Each PSUM tile spans 2 banks and holds 2 M-chunks, so one
            # PSUM->SBUF copy (which also downcasts to bf16) evacuates half
            # a pass: 4 copies per plane, split evenly ACT/DVE (GpSimd
            # cannot touch PSUM).
            def pass1(p, xt):
                zt = zpool.tile([P, A, W], bf16, name="zt", tag="zt")
                zts[p] = zt
                for half in range(2):
                    ps = ps1.tile([P, 2, W], f32, name="ps1", tag="ps1")
                    for j in range(2):
                        m = 2 * half + j
                        for a in range(A):
                            n0, n1 = win[a]
                            nc.tensor.matmul(
                                ps[:, j, n0:n1],
                                xt[:, a, m * P : (m + 1) * P],
                                bt[:, a, n0:n1],
                                start=(a == 0),
                                stop=(a == A - 1),
                                skip_group_check=True,
                            )
                    if half == 0:
                        nc.scalar.copy(out=zt[:, 0:2, :], in_=ps[:])
                    else:
                        nc.vector.tensor_copy(out=zt[:, 2:4, :], in_=ps[:])

            def pass2(p):
                zt = zts[p]
                ot = opool.tile([P, A, W], bf16, name="ot", tag="ot")
                for half in range(2):
                    ps = ps2.tile([P, 2, W], f32, name="ps2", tag="ps2")
                    for j in range(2):
                        m = 2 * half + j
                        for a in range(A):
                            n0, n1 = win[a]
                            nc.tensor.matmul(
                                ps[:, j, n0:n1],
                                zt[:, a, m * P : (m + 1) * P],
                                bt[:, a, n0:n1],
                                start=(a == 0),
                                stop=(a == A - 1),
                                skip_group_check=True,
                            )
                    if half == 0:
                        nc.scalar.copy(out=ot[:, 0:2, :], in_=ps[:])
                    else:
                        nc.vector.tensor_copy(out=ot[:, 2:4, :], in_=ps[:])
                # Output DMAs: bulk planes go whole-plane on GpSimd's
                # software DGE (the GpSimd engine is otherwise idle; keeps
                # the ACT/SP sequencers free).  The first and last two
                # planes go on the ACT HWDGE ring: early ones start the
                # output stream ~7us sooner (ACT is not yet saturated during
                # ramp-in), late ones drain faster than SWDGE at the tail.
                if p <= 1 or p == n_planes - 2:
                    nc.scalar.dma_start(out=y_ap[p], in_=ot[:])
                elif p == n_planes - 1:
                    nc.scalar.dma_start(out=y_ap[p, :, 0:2, :], in_=ot[:, 0:2, :])
                    nc.scalar.dma_start(out=y_ap[p, :, 2:4, :], in_=ot[:, 2:4, :])
                else:
                    nc.gpsimd.dma_start(out=y_ap[p], in_=ot[:])

            # software pipeline with one-plane skew: pass1(p) runs on the PE
            # while pass1(p-1)'s evacuations finish, so pass2(p-1) never
            # stalls the PE on the zt copies.
            for p in range(n_planes + 1):
                if p < n_planes:
                    if p == 0:
                        xt = xt0
                    else:
                        xt = xpool.tile([P, A, W], bf16, name="xt", tag="xt")
                        # one DMA queue sustains only ~210 B/ns, which is
                        # exactly the pipeline's input consumption rate;
                        # route every third plane via the ACT HWDGE ring so
                        # the SP queue has headroom and jitter cannot starve
                        # the PE.
                        if p % 3 == 2:
                            nc.scalar.dma_start(out=xt[:], in_=x_ap[p])
                        else:
                            nc.sync.dma_start(out=xt[:], in_=x_ap[p])
                    pass1(p, xt)
                if p >= 1:
                    pass2(p - 1)

    # Drop the preamble's GpSimd memsets of unused const tiles: Q7 memsets
    # cost ~µs each and gate the post-preamble all-engine barrier, delaying
    # kernel start.  Keep any const a later instruction actually reads.
    used = set()
    for bb in nc.main_func.blocks:
        for inst in bb.instructions:
            if type(inst).__name__ == "InstMemset":
                continue
            for ap in list(inst.ins or []) + list(inst.outs or []):
                ref = getattr(ap, "memref", None)
                if ref and str(ref).startswith("const-"):
                    used.add(str(ref))
    entry = nc.main_func.blocks[0]
    dropped = [
        inst
        for inst in entry.instructions
        if type(inst).__name__ == "InstMemset"
        and inst.outs
        and str(getattr(inst.outs[0], "memref", "")).startswith("const-")
        and str(inst.outs[0].memref) not in used
    ]
    for inst in dropped:
        entry.instructions.remove(inst)

    nc.finalize()
    return nc


def _box_matrix(r, dtype):
    inv_k = 1.0 / (2 * r + 1)
    b = np.zeros((H, W), dtype=np.float32)
    for i in range(H):
        b[i, max(0, i - r) : min(W, i + r + 1)] = inv_k
    return b.astype(dtype)


def kernel(x, r):
    import ml_dtypes
    from concourse.bass_utils import run_bass_kernel_spmd

    bf16 = ml_dtypes.bfloat16
    r = int(r)
    x = np.asarray(x, dtype=np.float32)
    n, c, h, w = x.shape
    assert (h, w) == (H, W) and n == N_CORES, (n, c, h, w)

    key = (r, c)
    if key not in _CACHE:
        _CACHE[key] = _build(r, c)
    nc = _CACHE[key]

    xb = np.ascontiguousarray(x.reshape(n, c * H, W)).astype(bf16)
    b = _box_matrix(r, bf16)
    in_maps = [{"x": xb[i], "b": b} for i in range(n)]
    res = run_bass_kernel_spmd(nc, in_maps, core_ids=list(range(N_CORES)))
    out = np.stack(
        [res.results[i]["y"].astype(np.float32).reshape(c, H, W) for i in range(n)]
    )
    return out


# revision 15
# speedup vs baseline: 1.1675x; 1.1675x over previous
"""Separable depthwise box filter (r=8, 'same' zero padding) on 8 trn2 cores.

Math: per (n, c) plane P (512x512), out = B @ P @ B where B is the symmetric
banded 512x512 matrix with B[i, j] = 1/(2r+1) for |i - j| <= r.  On the PE
(out = lhsT.T @ rhs):

  pass 1: Zt = matmul(lhsT=P,  rhs=B) = P.T @ B   (vertical filter, transposed)
  pass 2: Y  = matmul(lhsT=Zt, rhs=B) = Z  @ B    (horizontal filter, restored)

Both passes stream only the banded columns of B: the K-chunk of rows
[128a, 128a+128) of B has nonzero columns only in [128a-r, 128a+128+r).
PSUM's per-element has_written bit makes the overlapping column windows
accumulate while fresh columns overwrite, so each (M-chunk, K-chunk) pair is
a single matmul: 560 streamed columns per M-chunk instead of 2048.

Everything on-device is bf16 (fp32 matmul streams at 1/4 rate; the 2e-2
tolerance leaves ~40x headroom over bf16's quantization error).  The host
casts x -> bf16 and the bf16 result -> fp32.  PSUM accumulates in fp32; the
PSUM->SBUF evacuations (which also downcast) are spread over the DVE, ACT
and GpSimd engines.  The two passes are software-pipelined with a one-plane
skew (pass 1 of plane p runs on the PE while pass 2 of plane p-1's operand
tile is still being evacuated) so evacuation latency never bubbles the PE.

Sharding: batch dim (8) across the 8 cores; each core filters its 16 channel
planes independently (no cross-core communication).
"""

import numpy as np

_CACHE = {}

N_CORES = 8
P = 128
H = W = 512
A = H // P  # 4 row-chunks per plane


def _band_windows(r):
    """Nonzero column window [n0, n1) of B rows [128a, 128a+128), per a."""
    return [(max(0, P * a - r), min(W, P * a + P + r)) for a in range(A)]


def _build(r, n_planes):
    import concourse.mybir as mybir
    from concourse import bacc
    from concourse.tile import TileContext

    bf16 = mybir.dt.bfloat16
    f32 = mybir.dt.float32
    win = _band_windows(r)

    nc = bacc.Bacc()
    x_d = nc.declare_dram_parameter("x", [n_planes * H, W], bf16, isOutput=False)
    b_d = nc.declare_dram_parameter("b", [H, W], bf16, isOutput=False)
    y_d = nc.declare_dram_parameter("y", [n_planes * H, W], bf16, isOutput=True)

    x_ap = x_d.ap().rearrange("(p a q) n -> p q a n", p=n_planes, q=P)
    y_ap = y_d.ap().rearrange("(p a q) n -> p q a n", p=n_planes, q=P)
    b_ap = b_d.ap().rearrange("(a q) n -> q a n", q=P)

    with TileContext(nc) as tc:
        with (
            tc.tile_pool(name="bmat", bufs=1) as bpool,
            tc.tile_pool(name="xin", bufs=6) as xpool,
            tc.tile_pool(name="zmid", bufs=3) as zpool,
            tc.tile_pool(name="yout", bufs=6) as opool,
            tc.tile_pool(name="ps1", bufs=2, space="PSUM") as ps1,
            tc.tile_pool(name="ps2", bufs=2, space="PSUM") as ps2,
        ):
            bt = bpool.tile([P, A, W], bf16)
            xt0 = xpool.tile([P, A, W], bf16, name="xt0", tag="xt")
            # Interleave plane-0 x chunks with B chunks on the SP HWDGE ring
            # so the a=0 matmuls can start early; land the first matmul's
            # operands (B window 0 + x chunk 0) first.
            w0, w1 = win[0]
            nc.sync.dma_start(out=bt[:, 0, w0:w1], in_=b_ap[:, 0, w0:w1])
            nc.sync.dma_start(out=xt0[:, 0, :], in_=x_ap[0, :, 0, :])
            nc.sync.dma_start(out=bt[:, 0, w1:W], in_=b_ap[:, 0, w1:W])
            nc.sync.dma_start(out=xt0[:, 1, :], in_=x_ap[0, :, 1, :])
            nc.sync.dma_start(out=bt[:, 1, :], in_=b_ap[:, 1, :])
            nc.sync.dma_start(out=xt0[:, 2:4, :], in_=x_ap[0, :, 2:4, :])
            nc.sync.dma_start(out=bt[:, 2:4, :], in_=b_ap[:, 2:4, :])

            # Warm the PE while the input stream primes: the Tensor engine
            # needs ~3us of continuous execution to leave the low/mid
            # p-states (0.65/1.2 GHz) and reach 2.4 GHz.  Dummy matmuls on
            # the first-landed B chunk (result discarded) put the ramp time
            # behind us before the first real plane is ready.
            warm = ps1.tile([P, 2, W], f32, name="warm", tag="ps1")
            for _ in range(7):
                nc.tensor.matmul(
                    warm[:, 0, :],
                    bt[:, 0, 0:P],
                    bt[:, 0, :],
                    start=True,
                    stop=True,
                    skip_group_check=True,
                )

            zts = [None] * n_planes

            # Each PSUM tile spans 2 banks and holds 2 M-chunks, so one
            # PSUM->SBUF copy (which also downcasts to bf16) evacuates half
            # a pass: 4 copies per plane, split evenly ACT/DVE (GpSimd
            # cannot touch PSUM).
            def pass1(p, xt):
                zt = zpool.tile([P, A, W], bf16, name="zt", tag="zt")
                zts[p] = zt
                for half in range(2):
                    ps = ps1.tile([P, 2, W], f32, name="ps1", tag="ps1")
                    for j in range(2):
                        m = 2 * half + j
                        for a in range(A):
                            n0, n1 = win[a]
                            nc.tensor.matmul(
                                ps[:, j, n0:n1],
                                xt[:, a, m * P : (m + 1) * P],
                                bt[:, a, n0:n1],
                                start=(a == 0),
                                stop=(a == A - 1),
                                skip_group_check=True,
                            )
                    if half == 0:
                        nc.scalar.copy(out=zt[:, 0:2, :], in_=ps[:])
                    else:
                        nc.vector.tensor_copy(out=zt[:, 2:4, :], in_=ps[:])

            def pass2(p):
                zt = zts[p]
                ot = opool.tile([P, A, W], bf16, name="ot", tag="ot")
                for half in range(2):
                    ps = ps2.tile([P, 2, W], f32, name="ps2", tag="ps2")
                    for j in range(2):
                        m = 2 * half + j
                        for a in range(A):
                            n0, n1 = win[a]
                            nc.tensor.matmul(
                                ps[:, j, n0:n1],
                                zt[:, a, m * P : (m + 1) * P],
                                bt[:, a, n0:n1],
                                start=(a == 0),
                                stop=(a == A - 1),
                                skip_group_check=True,
                            )
                    if half == 0:
                        nc.scalar.copy(out=ot[:, 0:2, :], in_=ps[:])
                    else:
                        nc.vector.tensor_copy(out=ot[:, 2:4, :], in_=ps[:])
                # Output DMAs: bulk planes go whole-plane on GpSimd's
                # software DGE (the GpSimd engine is otherwise idle; keeps
                # the ACT/SP sequencers free).  The first and last two
                # planes go on the ACT HWDGE ring: early ones start the
                # output stream ~7us sooner (ACT is not yet saturated during
                # ramp-in), late ones drain faster than SWDGE at the tail.
                if p <= 1 or p == n_planes - 2:
                    nc.scalar.dma_start(out=y_ap[p], in_=ot[:])
                elif p == n_planes - 1:
                    nc.scalar.dma_start(out=y_ap[p, :, 0:2, :], in_=ot[:, 0:2, :])
                    nc.scalar.dma_start(out=y_ap[p, :, 2:4, :], in_=ot[:, 2:4, :])
                else:
                    nc.gpsimd.dma_start(out=y_ap[p], in_=ot[:])

            # software pipeline with one-plane skew: pass1(p) runs on the PE
            # while pass1(p-1)'s evacuations finish, so pass2(p-1) never
            # stalls the PE on the zt copies.
            for p in range(n_planes + 1):
                if p < n_planes:
                    if p == 0:
                        xt = xt0
                    else:
                        xt = xpool.tile([P, A, W], bf16, name="xt", tag="xt")
                        # all input on the SP ring: SP has no other duties,
                        # so its queue can afford to block on xt recycle.
                        # (Routing input via the ACT ring was tried and
                        # regressed 18%: ACT's in-order sequencer stalls its
                        # evacuation copies behind input gens waiting on
                        # buffer recycling.)
                        nc.sync.dma_start(out=xt[:], in_=x_ap[p])
                    pass1(p, xt)
                if p >= 1:
                    pass2(p - 1)

    # Drop the preamble's GpSimd memsets of unused const tiles: Q7 memsets
    # cost ~µs each and gate the post-preamble all-engine barrier, delaying
    # kernel start.  Keep any const a later instruction actually reads.
    used = set()
    for bb in nc.main_func.blocks:
        for inst in bb.instructions:
            if type(inst).__name__ == "InstMemset":
                continue
            for ap in list(inst.ins or []) + list(inst.outs or []):
                ref = getattr(ap, "memref", None)
                if ref and str(ref).startswith("const-"):
                    used.add(str(ref))
    entry = nc.main_func.blocks[0]
    dropped = [
        inst
        for inst in entry.instructions
        if type(inst).__name__ == "InstMemset"
        and inst.outs
        and str(getattr(inst.outs[0], "memref", "")).startswith("const-")
        and str(inst.outs[0].memref) not in used
    ]
    for inst in dropped:
        entry.instructions.remove(inst)

    nc.finalize()
    return nc


def _box_matrix(r, dtype):
    inv_k = 1.0 / (2 * r + 1)
    b = np.zeros((H, W), dtype=np.float32)
    for i in range(H):
        b[i, max(0, i - r) : min(W, i + r + 1)] = inv_k
    return b.astype(dtype)


def kernel(x, r):
    import ml_dtypes
    from concourse.bass_utils import run_bass_kernel_spmd

    bf16 = ml_dtypes.bfloat16
    r = int(r)
    x = np.asarray(x, dtype=np.float32)
    n, c, h, w = x.shape
    assert (h, w) == (H, W) and n == N_CORES, (n, c, h, w)

    key = (r, c)
    if key not in _CACHE:
        _CACHE[key] = _build(r, c)
    nc = _CACHE[key]

    xb = np.ascontiguousarray(x.reshape(n, c * H, W)).astype(bf16)
    b = _box_matrix(r, bf16)
    in_maps = [{"x": xb[i], "b": b} for i in range(n)]
    res = run_bass_kernel_spmd(nc, in_maps, core_ids=list(range(N_CORES)))
    out = np.stack(
        [res.results[i]["y"].astype(np.float32).reshape(c, H, W) for i in range(n)]
    )
    return out
